# revision 4
# baseline (speedup 1.0000x reference)
"""Trainium2 Bass kernel for nn_CombineConcat (pairwise broadcast+concat).

reference semantics (per batch b):
  out[b, i*N + j, 0:D]   = x1[b, i, :]
  out[b, i*N + j, D:2*D] = x2[b, j, :]

Shapes (hardcoded): x1, x2 = [16, 128, 256] f32 -> out = [16, 16384, 512] f32.

Strategy: data-parallel over the batch dim, 2 batches per core on 8 cores.
Write-bandwidth bound: each core writes 64 MB (reads 512 KB). The 16 SDMA
engines sustain ~380 GB/s with 2KB-per-partition descriptors, so the data
floor is ~178 us; everything else must hide behind the write stream.

v2 changes vs the 207us baseline (which idled ~20us/engine at ramp and the
batch boundary):
 - x1 rows are staged through a 4-deep partition-0 ping-pong pool, one
   small DRAM load per broadcast group (pbcast sources must start at
   partition 0 - BIR verifier rule). No monolithic x1flat staging, so
   there is no mid-run reload and batch-1 prefetch overlaps batch-0.
 - 64-slot ring (128 KB/partition) so gpsimd broadcast runs far ahead of
   the DMA stream and batch-1 broadcasts/refills overlap batch-0's tail.
 - one DMA descriptor per broadcast group (up to 8 blocks = 2 MB, 3-dim
   access pattern) instead of per block: descriptor issue drops from
   ~74 us/engine/batch (marginal vs the 89 us drain time) to ~20 us.
 - pbcast group schedule [2,2,4,8...]: small groups first for fast ramp.
"""

import numpy as np

_B, _N, _D = 16, 128, 256
_NCORES = 8
_BPC = _B // _NCORES  # batches per core

_NC_CACHE = {}


def _build_nc(bpc=_BPC, n=_N, d=_D, k_ring=64, stage_bufs=4):
    import concourse.bacc as bacc
    import concourse.mybir as mybir
    from concourse.tile import TileContext

    f32 = mybir.dt.float32
    nc = bacc.Bacc("TRN2", target_bir_lowering=False, enable_partition_id=False)
    x1 = nc.dram_tensor("x1", [bpc, n, d], f32, kind="ExternalInput")
    x2 = nc.dram_tensor("x2", [bpc, n, d], f32, kind="ExternalInput")
    out = nc.dram_tensor("out", [bpc, n * n, 2 * d], f32, kind="ExternalOutput")
    W = 2 * d  # ring slot width in elements

    # group schedule: (block_start, group_size). Small groups first so the
    # first output descriptors exist ASAP; 8-block groups in steady state.
    sched = [(0, 2), (2, 2), (4, 4)] + [(8 + 8 * j, 8) for j in range((n - 8) // 8)]
    assert sum(g for _, g in sched) == n
    Gmax = max(g for _, g in sched)
    x1f = [x1[b].rearrange("n d -> (n d)") for b in range(bpc)]

    with TileContext(nc) as tc:
        with (
            tc.tile_pool(name="io", bufs=1) as iop,
            tc.tile_pool(name="x1s", bufs=stage_bufs) as spool,
            tc.tile_pool(name="ring", bufs=1) as rp,
        ):
            t2s = [
                iop.tile([n, d], f32, tag=f"t2_{b}", name=f"t2_{b}")
                for b in range(bpc)
            ]

            def stage_load(b, gi):
                """DMA x1[b] rows [i0, i0+G) flat onto a partition-0 stage."""
                i0, G = sched[gi]
                s = spool.tile(
                    [1, Gmax * d], f32, tag="x1s", name=f"x1s_{b}_{gi}"
                )
                eng = nc.sync if gi % 2 == 0 else nc.scalar
                eng.dma_start(
                    out=s[0:1, 0 : G * d], in_=x1f[b][i0 * d : (i0 + G) * d]
                )
                return s

            # ring: slot k holds [x1_i | x2] for block i (k = i % k_ring)
            RB = rp.tile([n, k_ring * W], f32, tag="RB")
            RBv = RB[:].rearrange("p (k h c) -> p k h c", k=k_ring, c=d)

            stages = {}
            for gi in range(stage_bufs):
                stages[(0, gi)] = stage_load(0, gi)
            nc.scalar.dma_start(out=t2s[0][:], in_=x2[0])

            for b in range(bpc):
                if b > 0:
                    nc.scalar.dma_start(out=t2s[b][:], in_=x2[b])
                ob = out[b]  # [n*n, 2d]
                for gi, (i0, G) in enumerate(sched):
                    k0 = i0 % k_ring
                    # x2 halves: written once per batch per slot (slots are
                    # reused within a batch with identical x2 content).
                    if i0 < k_ring:
                        for k in range(k0, k0 + G):
                            nc.vector.tensor_copy(out=RBv[:, k, 1, :], in_=t2s[b][:])
                    # x1 halves: broadcast rows i0..i0+G-1 from the stage.
                    nc.gpsimd.partition_broadcast(
                        RBv[:, k0 : k0 + G, 0, :],
                        stages.pop((b, gi))[0:1, 0 : G * d],
                        opt=False,
                    )
                    # one descriptor for the whole group: src [128, G*512],
                    # dst rows i0*128..(i0+G)*128 iterated (p, g, c).
                    eng = nc.scalar if gi % 2 == 0 else nc.sync
                    eng.dma_start(
                        out=ob[i0 * n : (i0 + G) * n, :].rearrange(
                            "(g p) c -> p g c", g=G
                        ),
                        in_=RB[:, k0 * W : (k0 + G) * W],
                    )
                    # prefetch the stage 4 groups ahead (rolls into batch b+1)
                    ng = gi + stage_bufs
                    if ng < len(sched):
                        stages[(b, ng)] = stage_load(b, ng)
                    elif b + 1 < bpc:
                        ng -= len(sched)
                        stages[(b + 1, ng)] = stage_load(b + 1, ng)
    nc.finalize()
    return nc


def _get_nc():
    if "nc" not in _NC_CACHE:
        _NC_CACHE["nc"] = _build_nc()
    return _NC_CACHE["nc"]


def _run(x1, x2, trace=False):
    """Run the kernel on 8 cores; returns (output, BassKernelResults)."""
    from concourse.bass_utils import run_bass_kernel_spmd

    nc = _get_nc()
    x1 = np.ascontiguousarray(np.asarray(x1, dtype=np.float32))
    x2 = np.ascontiguousarray(np.asarray(x2, dtype=np.float32))
    in_maps = [
        {
            "x1": x1[c * _BPC : (c + 1) * _BPC],
            "x2": x2[c * _BPC : (c + 1) * _BPC],
        }
        for c in range(_NCORES)
    ]
    res = run_bass_kernel_spmd(
        nc, in_maps, core_ids=list(range(_NCORES)), trace=trace
    )
    out = np.concatenate([r["out"] for r in res.results], axis=0)
    return out, res


def kernel(x1, x2):
    out, _ = _run(x1, x2, trace=False)
    return out
